# revision 3
# baseline (speedup 1.0000x reference)
"""Trainium2 Bass kernel v2 for nn_Attention_40372692582854.

Single-head attention block, data-parallel over batch (1 batch elem/core).

Key changes vs v1: x is host-transposed (xT [D,S] bf16 sent alongside x
[S,D] f32 for the residual), so the 96 PE transposes and bn_stats LN path
are gone.  LN stats come from ones-matmuls on PE (sum and sum-of-squares
over the partition/d axis), the mean correction and all QKV biases ride the
PSUM accumulations as K=2 rank-1 matmuls, and rstd is folded into x1T once.

Per-core dataflow (S=2048, D=768, chunks of 512 tokens):
  Phase A (per chunk c):
    xT tiles [128,512]  <- DMA (bf16)
    xsq = Square(xT)  (ACT)
    pstat[2,512] = [ones.T @ xT ; ones.T @ xsq]   (PE, 12 MMs)
    mu_neg/e2/var rows (DVE), rstd row = Rsqrt(var+eps) (ACT, bf16)
    aff[1,chunk] = mu_neg * rstd  (the -mu*rstd rank-1 row; aff[0]=ones)
    rstd_b [128,512] = ones1.T @ rstd_row (PE) -> SBUF copy (ACT)
    x1T[j] = xT[j] * rstd_b  (DVE, bf16)
    v[t]  = x1T.T @ wv  (+ rank-1: ones x bias_v + negmurstd x colsum_wv)
    kT[j], qT[j] chunk = wqk.T @ x1T (+ rank-1: bias + colsum x negmurstd)
  Phase B (per chunk c):
    scoresT[k,512] = kT.T @ qT_chunk ; pT = exp(scoresT)  (no max-sub)
    den[1,512] accumulated via ones-matmuls; reciprocal via DRAM bounce
    outT[ot,512] = v.T @ pT
    y[t] = gelu((outT.T @ wo) * inv_den + b_out + x)  -> DMA out
"""

import numpy as np
import ml_dtypes
from contextlib import ExitStack

import concourse.bass as bass
import concourse.tile as tile
import concourse.mybir as mybir
from concourse import bacc
from concourse.bass_utils import run_bass_kernel_spmd

F32 = mybir.dt.float32
BF16 = mybir.dt.bfloat16
AF = mybir.ActivationFunctionType
OP = mybir.AluOpType

B = 8
S = 2048
D = 768
P = 128
DT = D // P            # 6 dim tiles
ST = S // P            # 16 token tiles
SC = 512               # chunk width (tokens)
NSC = S // SC          # 4 chunks
TPC = SC // P          # 4 token tiles per chunk
EPS = 1e-5


def ts(i, n):
    return bass.ts(i, n)


def build_bass(reps=1):
    nc = bacc.Bacc("TRN2")

    xT_d = nc.dram_tensor("xT", [D, S], BF16, kind="ExternalInput")
    x_d = nc.dram_tensor("x", [S, D], F32, kind="ExternalInput")
    wqk_d = nc.dram_tensor("wqk", [D, 2 * D], BF16, kind="ExternalInput")
    wv_d = nc.dram_tensor("wv", [D, D], BF16, kind="ExternalInput")
    wo_d = nc.dram_tensor("wo", [D, D], BF16, kind="ExternalInput")
    affw_d = nc.dram_tensor("affw", [2, 2 * D], BF16, kind="ExternalInput")
    affv_d = nc.dram_tensor("affv", [2, D], BF16, kind="ExternalInput")
    bo_d = nc.dram_tensor("bo", [P, D], F32, kind="ExternalInput")
    out_d = nc.dram_tensor("out", [S, D], F32, kind="ExternalOutput")

    with tile.TileContext(nc) as tc:
      _ccm = tc.tile_pool(name="const", bufs=1)
      const = _ccm.__enter__()
      # weights/constants loaded once per NEFF; reps keep them resident
      wqk_t = [const.tile([P, 2 * D], BF16, tag=f"wqk{i}", name=f"wqk{i}")
               for i in range(DT)]
      wv_t = [const.tile([P, D], BF16, tag=f"wv{i}", name=f"wv{i}")
              for i in range(DT)]
      wo_t = [const.tile([P, D], BF16, tag=f"wo{i}", name=f"wo{i}")
              for i in range(DT)]
      affw_t = const.tile([2, 2 * D], BF16, tag="affw", name="affw")
      affv_t = const.tile([2, D], BF16, tag="affv", name="affv")
      bo_t = const.tile([P, D], F32, tag="bo", name="bo")
      for i in range(DT):
          nc.gpsimd.dma_start(out=wv_t[i], in_=wv_d[ts(i, P), :])
      for i in range(DT):
          nc.gpsimd.dma_start(out=wqk_t[i], in_=wqk_d[ts(i, P), :])
      nc.gpsimd.dma_start(out=affw_t, in_=affw_d[:, :])
      nc.gpsimd.dma_start(out=affv_t, in_=affv_d[:, :])
      for i in range(DT):
          nc.gpsimd.dma_start(out=wo_t[i], in_=wo_d[ts(i, P), :])
      nc.gpsimd.dma_start(out=bo_t, in_=bo_d[:, :])
      onesK = const.tile([P, 1], BF16, tag="onesK", name="onesK")
      nc.vector.memset(onesK, 1.0)
      ones1 = const.tile([1, P], BF16, tag="ones1", name="ones1")
      nc.vector.memset(ones1, 1.0)
      eps_t = const.tile([1, 1], F32, tag="eps", name="eps")
      nc.vector.memset(eps_t, EPS)
      for _rep in range(reps):
        with ExitStack() as ctx:
          big = ctx.enter_context(tc.tile_pool(name="big", bufs=1))

          # ---- persistent activations ----
          v_t = [big.tile([P, D], BF16, tag=f"v{t}", name=f"v{t}")
                 for t in range(ST)]
          kT = [big.tile([P, S], BF16, tag=f"kT{j}", name=f"kT{j}")
                for j in range(DT)]
          qT = [big.tile([P, S], BF16, tag=f"qT{j}", name=f"qT{j}")
                for j in range(DT)]
          aff = big.tile([2, S], BF16, tag="aff", name="aff")
          nc.vector.memset(aff[0:1, :], 1.0)
          inv_den = big.tile([P, ST], F32, tag="inv_den", name="inv_den")

          # ================= Phase A: LN stats + V/K/Q projections ==========
          with tc.tile_pool(name="lnp", bufs=2) as lnp, \
               tc.tile_pool(name="rowp", bufs=2) as rowp, \
               tc.tile_pool(name="x1p", bufs=2) as x1p, \
               tc.tile_pool(name="pstat", bufs=1, space="PSUM") as pstat, \
               tc.tile_pool(name="pbc", bufs=1, space="PSUM") as pbc, \
               tc.tile_pool(name="pv", bufs=2, space="PSUM") as pvp, \
               tc.tile_pool(name="pkq", bufs=2, space="PSUM") as pkq:
            for c in range(NSC):
              xTt = []
              xsq = []
              for j in range(DT):
                  xt = lnp.tile([P, SC], BF16, tag=f"xt{j}", name=f"xt{j}")
                  nc.sync.dma_start(out=xt, in_=xT_d[ts(j, P), ts(c, SC)])
                  xTt.append(xt)
                  sq = lnp.tile([P, SC], BF16, tag=f"sq{j}", name=f"sq{j}")
                  nc.scalar.activation(out=sq, in_=xt, func=AF.Square)
                  xsq.append(sq)
              pst = pstat.tile([33, SC], F32, tag="pst", name="pst")
              for j in range(DT):
                  nc.tensor.matmul(pst[0:1, :], lhsT=onesK, rhs=xTt[j],
                                   start=(j == 0), stop=(j == DT - 1))
              for j in range(DT):
                  nc.tensor.matmul(pst[32:33, :], lhsT=onesK, rhs=xsq[j],
                                   start=(j == 0), stop=(j == DT - 1))
              # rows: mu_neg = -sum/D ; e2 = sumsq/D
              me0 = rowp.tile([1, SC], F32, tag="me0", name="me0")
              nc.vector.tensor_scalar(out=me0, in0=pst[0:1, :],
                                      scalar1=-1.0 / D, scalar2=None,
                                      op0=OP.mult)
              me1 = rowp.tile([1, SC], F32, tag="me1", name="me1")
              nc.vector.tensor_scalar(out=me1, in0=pst[32:33, :],
                                      scalar1=1.0 / D, scalar2=None,
                                      op0=OP.mult)
              musq = rowp.tile([1, SC], F32, tag="musq", name="musq")
              nc.vector.tensor_tensor(out=musq, in0=me0, in1=me0,
                                      op=OP.mult)
              varr = rowp.tile([1, SC], F32, tag="varr", name="varr")
              nc.vector.tensor_tensor(out=varr, in0=me1, in1=musq,
                                      op=OP.subtract)
              std_row = rowp.tile([1, SC], F32, tag="std", name="std")
              nc.scalar.activation(out=std_row, in_=varr, func=AF.Sqrt,
                                   bias=eps_t, scale=1.0)
              rstd_row = rowp.tile([1, SC], BF16, tag="rstd", name="rstd")
              with nc.allow_low_precision(reason="rstd is bf16 by design"):
                  nc.vector.reciprocal(out=rstd_row, in_=std_row)
              negmr = rowp.tile([1, SC], BF16, tag="negmr", name="negmr")
              nc.vector.tensor_tensor(out=negmr, in0=me0,
                                      in1=rstd_row, op=OP.mult)
              # DVE cannot write partition 1; bounce via SBUF->SBUF DMA
              nc.sync.dma_start(out=aff[1:2, ts(c, SC)], in_=negmr)
              # broadcast rstd over partitions via K=1 matmul
              pbct = pbc.tile([P, SC], F32, tag="pbct", name="pbct")
              nc.tensor.matmul(pbct, lhsT=ones1, rhs=rstd_row,
                               start=True, stop=True)
              rstd_b = rowp.tile([P, SC], BF16, tag="rstd_b", name="rstd_b")
              nc.scalar.copy(out=rstd_b, in_=pbct)
              x1T = []
              for j in range(DT):
                  x1 = x1p.tile([P, SC], BF16, tag=f"x1{j}", name=f"x1{j}")
                  nc.vector.tensor_tensor(out=x1, in0=xTt[j], in1=rstd_b,
                                          op=OP.mult)
                  x1T.append(x1)

              # ---- V projection for the 4 token tiles of this chunk ----
              for tt in range(TPC):
                  t = c * TPC + tt
                  psv = pvp.tile([P, D], F32, tag="psv", name="psv")
                  for h0, hn in ((0, 512), (512, 256)):
                      for j in range(DT):
                          nc.tensor.matmul(
                              psv[:, h0:h0 + hn],
                              lhsT=x1T[j][:, ts(tt, P)],
                              rhs=wv_t[j][:, h0:h0 + hn],
                              start=(j == 0), stop=False)
                      nc.tensor.matmul(
                          psv[:, h0:h0 + hn],
                          lhsT=aff[:, ts(t, P)],
                          rhs=affv_t[:, h0:h0 + hn],
                          start=False, stop=True)
                  nc.scalar.copy(out=v_t[t], in_=psv)

              # ---- K and Q projections for this chunk ----
              for which, dst in ((1, kT), (0, qT)):
                  for j in range(DT):
                      w0 = which * D + j * P
                      pso = pkq.tile([P, SC], F32, tag="pso", name="pso")
                      for dt in range(DT):
                          nc.tensor.matmul(
                              pso,
                              lhsT=wqk_t[dt][:, w0:w0 + P],
                              rhs=x1T[dt],
                              start=(dt == 0), stop=False)
                      nc.tensor.matmul(
                          pso,
                          lhsT=affw_t[:, w0:w0 + P],
                          rhs=aff[:, ts(c, SC)],
                          start=False, stop=True)
                      nc.scalar.copy(out=dst[j][:, ts(c, SC)], in_=pso)

          # ================= Phase B: attention + output ====================
          with tc.tile_pool(name="att", bufs=2) as att, \
               tc.tile_pool(name="att2", bufs=2) as att2, \
               tc.tile_pool(name="dram", bufs=2, space="DRAM") as dram, \
               tc.tile_pool(name="patt", bufs=3, space="PSUM") as patt, \
               tc.tile_pool(name="po", bufs=2, space="PSUM") as pop, \
               tc.tile_pool(name="pden", bufs=1, space="PSUM") as pdenp, \
               tc.tile_pool(name="py", bufs=1, space="PSUM") as pyp:
            for c in range(NSC):
              pT = [att.tile([P, SC], BF16, tag=f"pT{kt}", name=f"pT{kt}")
                    for kt in range(ST)]
              ps_den = pdenp.tile([1, SC], F32, tag="pden", name="pden")
              for kt in range(ST):
                  ps_s = patt.tile([P, SC], F32, tag="ps_s", name="ps_s")
                  for j in range(DT):
                      nc.tensor.matmul(ps_s,
                                       lhsT=kT[j][:, ts(kt, P)],
                                       rhs=qT[j][:, ts(c, SC)],
                                       start=(j == 0), stop=(j == DT - 1))
                  nc.scalar.activation(out=pT[kt], in_=ps_s, func=AF.Exp)
                  nc.tensor.matmul(ps_den, lhsT=onesK, rhs=pT[kt],
                                   start=(kt == 0), stop=(kt == ST - 1))

              den_row = att2.tile([1, SC], F32, tag="den_row", name="den_row")
              nc.vector.tensor_copy(out=den_row, in_=ps_den)
              den_b = dram.tile([1, SC], F32, tag="den_b", name="den_b")
              nc.sync.dma_start(out=den_b, in_=den_row)
              den_pp = att2.tile([P, TPC], F32, tag="den_pp", name="den_pp")
              nc.sync.dma_start(out=den_pp,
                                in_=den_b.rearrange("a (t p) -> (a p) t", p=P))
              nc.vector.reciprocal(out=inv_den[:, c * TPC:(c + 1) * TPC],
                                   in_=den_pp)

              outT = []
              for ot in range(DT):
                  ps_o = pop.tile([P, SC], F32, tag="ps_o", name="ps_o")
                  for kt in range(ST):
                      nc.tensor.matmul(ps_o,
                                       lhsT=v_t[kt][:, ts(ot, P)],
                                       rhs=pT[kt],
                                       start=(kt == 0), stop=(kt == ST - 1))
                  ob = att2.tile([P, SC], BF16, tag=f"outT{ot}",
                                 name=f"outT{ot}")
                  nc.scalar.copy(out=ob, in_=ps_o)
                  outT.append(ob)

              for tt in range(TPC):
                  t = c * TPC + tt
                  ps_y = pyp.tile([P, D], F32, tag="ps_y", name="ps_y")
                  for h0, hn in ((0, 512), (512, 256)):
                      for ot in range(DT):
                          nc.tensor.matmul(
                              ps_y[:, h0:h0 + hn],
                              lhsT=outT[ot][:, ts(tt, P)],
                              rhs=wo_t[ot][:, h0:h0 + hn],
                              start=(ot == 0), stop=(ot == DT - 1))
                  xr = att2.tile([P, D], F32, tag="xr", name="xr")
                  nc.sync.dma_start(out=xr, in_=x_d[ts(t, P), :])
                  xb = att2.tile([P, D], F32, tag="xb", name="xb")
                  nc.gpsimd.tensor_tensor(out=xb, in0=xr, in1=bo_t, op=OP.add)
                  t1 = att2.tile([P, D], F32, tag="t1", name="t1")
                  nc.vector.tensor_scalar(out=t1, in0=ps_y,
                                          scalar1=inv_den[:, t:t + 1],
                                          scalar2=None, op0=OP.mult)
                  y_t = att2.tile([P, D], F32, tag="y_t", name="y_t")
                  nc.vector.tensor_tensor(out=y_t, in0=t1, in1=xb, op=OP.add)
                  g_t = att2.tile([P, D], F32, tag="g_t", name="g_t")
                  nc.scalar.activation(out=g_t, in_=y_t, func=AF.Gelu)
                  nc.gpsimd.dma_start(out=out_d[ts(t, P), :], in_=g_t)

      _ccm.__exit__(None, None, None)

    nc.compile()
    return nc


_NC_CACHE = None


def _get_nc():
    global _NC_CACHE
    if _NC_CACHE is None:
        _NC_CACHE = build_bass()
    return _NC_CACHE


def prep_inputs(x, ln_gamma, ln_beta, w_qkv, b_qkv, w_out, b_out):
    """Host-side weight prep; returns per-core in_maps."""
    x = np.asarray(x, np.float32)
    g = np.asarray(ln_gamma, np.float32)
    be = np.asarray(ln_beta, np.float32)
    w_qkv = np.asarray(w_qkv, np.float32)
    b_qkv = np.asarray(b_qkv, np.float32)
    w_out = np.asarray(w_out, np.float32)
    b_out = np.asarray(b_out, np.float32)

    sc = D ** -0.5
    wg = w_qkv * g[:, None]
    bias = be @ w_qkv + b_qkv
    wqk = np.concatenate([wg[:, :D] * sc, wg[:, D:2 * D]], axis=1)
    bqk = np.concatenate([bias[:D] * sc, bias[D:2 * D]])
    wv = wg[:, 2 * D:]
    bv = bias[2 * D:]
    affw = np.stack([bqk, wqk.sum(axis=0)])          # [2, 2D]
    affv = np.stack([bv, wv.sum(axis=0)])            # [2, D]
    shared = {
        "wqk": wqk.astype(ml_dtypes.bfloat16),
        "wv": wv.astype(ml_dtypes.bfloat16),
        "wo": w_out.astype(ml_dtypes.bfloat16),
        "affw": affw.astype(ml_dtypes.bfloat16),
        "affv": affv.astype(ml_dtypes.bfloat16),
        "bo": np.ascontiguousarray(np.broadcast_to(b_out, (P, D))),
    }
    return [dict(shared,
                 x=np.ascontiguousarray(x[b]),
                 xT=np.ascontiguousarray(x[b].T).astype(ml_dtypes.bfloat16))
            for b in range(B)]


def kernel(**inputs) -> np.ndarray:
    nc = _get_nc()
    in_maps = prep_inputs(**inputs)
    res = run_bass_kernel_spmd(nc, in_maps, core_ids=list(range(B)))
    return np.stack([res.results[b]["out"] for b in range(B)])
